# revision 3
# baseline (speedup 1.0000x reference)
"""Chamfer loss kernel for 8 trn2 NeuronCores.

Strategy:
  - core c handles (batch b = c//2, predict-half h = c%2):
      P = predict[b, :, h*4096:(h+1)*4096]  [3, 4096]
      G = gt[b]                             [3, 8192]
  - d2[m, n] = |P_n|^2 + |G_m|^2 - 2 P_n . G_m is produced by ONE K=5 matmul:
      lhsT rows (stationary, gt side):  [G0, G1, G2, 1,  g2]
      rhs  rows (moving, predict side): [-2P0, -2P1, -2P2, p2, 1]
    so each [128, 512] PSUM tile is a fully-formed block of squared distances.
  - On-chip reductions (sqrt is monotonic -> only mins needed, sqrt on host):
      z   (min over n per gt point m):   free-dim reduce_min per tile,
                                         accumulated -> z_mins [128, 64]
      z2  (min over m per predict pt n): elementwise min accumulate
                                         -> z2_acc [128, 4096]
  - Host: min-combine core pairs / partition axis, sqrt, sum, divide.
"""

import os
import sys

import numpy as np

_TRN_REPO = "/opt/trn_rl_repo"
if _TRN_REPO not in sys.path:
    sys.path.insert(0, _TRN_REPO)

import concourse.bass as bass
from concourse import bacc
import concourse.mybir as mybir
import concourse.tile as tile
from concourse.bass import ts
from concourse.bass_utils import run_bass_kernel_spmd

B = 4
C = 3
NP_FULL = 8192   # predict points per batch
NG = 8192        # gt points per batch
N_CORES = 8
NP_LOC = NP_FULL // 2          # predict points per core (4096)
K = 5                          # matmul contraction rows
MT = 128                       # m-tile: gt points per output-partition tile
NB = 512                       # n-block: predict points per free-dim block
N_MTILES = NG // MT            # 64
N_NBLKS = NP_LOC // NB         # 8
BIG = 3.0e38
EPS = 1e-12

LAST_EXEC_NS = None
_CACHE = {}


def _build():
    if "nc" in _CACHE:
        return _CACHE["nc"]
    nc = bacc.Bacc()
    f32 = mybir.dt.float32
    stat_in = nc.dram_tensor("stat_in", [K, NG + NP_LOC], f32, kind="ExternalInput")
    z_out = nc.dram_tensor("z_out", [MT, N_MTILES], f32, kind="ExternalOutput")
    z2_out = nc.dram_tensor("z2_out", [MT, NP_LOC], f32, kind="ExternalOutput")

    MIN = mybir.AluOpType.min
    AX = mybir.AxisListType.X

    with tile.TileContext(nc) as tc:
        with (
            tc.tile_pool(name="stat", bufs=1) as stat_pool,
            tc.tile_pool(name="psum", bufs=8, space="PSUM") as psum_pool,
            tc.tile_pool(name="zp", bufs=4) as zp_pool,
        ):
            stat_sb = stat_pool.tile([K, NG + NP_LOC], f32)
            nc.sync.dma_start(out=stat_sb, in_=stat_in[:, :])
            gt_sb = stat_sb[:, 0:NG]
            pr_sb = stat_sb[:, NG : NG + NP_LOC]

            z2_acc = stat_pool.tile([MT, NP_LOC], f32)
            nc.vector.memset(z2_acc, BIG)
            z_mins = stat_pool.tile([MT, N_MTILES], f32)

            for i in range(N_MTILES):
                z_parts = zp_pool.tile([MT, N_NBLKS], f32)
                for j in range(N_NBLKS):
                    ps = psum_pool.tile([MT, NB], f32)
                    nc.tensor.matmul(
                        ps,
                        gt_sb[:, ts(i, MT)],
                        pr_sb[:, ts(j, NB)],
                        start=True,
                        stop=True,
                    )
                    nc.vector.tensor_reduce(
                        z_parts[:, j : j + 1], ps, axis=AX, op=MIN
                    )
                    nc.vector.tensor_tensor(
                        z2_acc[:, ts(j, NB)], ps, z2_acc[:, ts(j, NB)], op=MIN
                    )
                nc.vector.tensor_reduce(z_mins[:, i : i + 1], z_parts, axis=AX, op=MIN)

            nc.sync.dma_start(out=z_out[:, :], in_=z_mins)
            nc.sync.dma_start(out=z2_out[:, :], in_=z2_acc)

    nc.compile()
    _CACHE["nc"] = nc
    return nc


def _prep_core_inputs(predict_pc, gt_pc, c):
    b, h = divmod(c, 2)
    P = predict_pc[b][:, h * NP_LOC : (h + 1) * NP_LOC].astype(np.float32)
    G = gt_pc[b].astype(np.float32)
    g2 = (G * G).sum(axis=0, keepdims=True)
    p2 = (P * P).sum(axis=0, keepdims=True)
    gt_stat = np.concatenate([G, np.ones((1, NG), np.float32), g2], axis=0)
    pr_mov = np.concatenate([-2.0 * P, p2, np.ones((1, NP_LOC), np.float32)], axis=0)
    stat = np.concatenate([gt_stat, pr_mov], axis=1)
    return {"stat_in": np.ascontiguousarray(stat, dtype=np.float32)}


def kernel(predict_pc, gt_pc):
    global LAST_EXEC_NS
    predict_pc = np.asarray(predict_pc, dtype=np.float32)
    gt_pc = np.asarray(gt_pc, dtype=np.float32)

    nc = _build()
    in_maps = [_prep_core_inputs(predict_pc, gt_pc, c) for c in range(N_CORES)]
    trace = os.environ.get("CHAMFER_TRACE", "0") == "1"
    res = run_bass_kernel_spmd(
        nc, in_maps, core_ids=list(range(N_CORES)), trace=trace
    )
    LAST_EXEC_NS = res.exec_time_ns

    denom = B * (NG + NP_FULL)
    z_sum = 0.0
    z2_sum = 0.0
    for b in range(B):
        r0 = res.results[2 * b]
        r1 = res.results[2 * b + 1]
        zmin = np.minimum(r0["z_out"], r1["z_out"])
        z_sum += np.sqrt(np.maximum(zmin, EPS)).sum(dtype=np.float64)
        for r in (r0, r1):
            z2 = r["z2_out"].min(axis=0)
            z2_sum += np.sqrt(np.maximum(z2, EPS)).sum(dtype=np.float64)
    loss = (z_sum + z2_sum) / denom
    return np.float32(loss)


# revision 4
# speedup vs baseline: 1.0670x; 1.0670x over previous
"""Chamfer loss kernel for 8 trn2 NeuronCores (final).

Sharding: core c = (batch b = c//2, predict-half h = c%2); each core
computes its [8192 gt x 4096 predict] squared-distance block and local
min-reductions; host does the tiny cross-core min-combine + sqrt + sum.

Pipeline (per core) + copy-all-bf16 + bf16 TT min-tree (no TTR).

Per m-tile i (128 gt points x 4096 local predict points):
  - PE: 8 fp32r matmuls -> two 4-bank psum tiles [128, 4, 512].
  - ACT: 2 copies psum -> cp [128, 4096] bf16.
  - VE z2: ONE big bf16 tensor_tensor (2x): z2_acc = min(cp, z2_acc).
  - VE z: bf16 TT min-tree (2x) + final 1x reduce:
        t1[0:2048] = min(cp[0:2048], cp[2048:4096])
        t2[0:1024] = min(t1[0:1024], t1[1024:2048])
        t3[0:512]  = min(t2[0:512],  t2[512:1024])
        z_mins[:, i] = reduce_min(t3)   (fp32 out)
Host: min-combine core pairs / partition axis, sqrt, sum.
"""

import os
import sys

import numpy as np

_TRN_REPO = "/opt/trn_rl_repo"
if _TRN_REPO not in sys.path:
    sys.path.insert(0, _TRN_REPO)

import concourse.bass as bass
from concourse import bacc
import concourse.mybir as mybir
import concourse.tile as tile
from concourse.bass import ts
from concourse.bass_utils import run_bass_kernel_spmd

B = 4
C = 3
NP_FULL = 8192
NG = 8192
N_CORES = 8
NP_LOC = NP_FULL // 2
K = 24
MT = 128
NB = 512
N_MTILES = NG // MT            # 64
N_NBLKS = NP_LOC // NB         # 8
HALF = N_NBLKS // 2            # 4
RGRP = 8                       # m-tiles per grouped z reduce
BIG = 3.0e38
EPS = 1e-12

LAST_EXEC_NS = None
_CACHE = {}


def _build():
    if "nc" in _CACHE:
        return _CACHE["nc"]
    nc = bacc.Bacc()
    f32 = mybir.dt.float32
    f32r = mybir.dt.float32r
    bf16 = mybir.dt.bfloat16
    stat_in = nc.dram_tensor("stat_in", [K, NG + NP_LOC], bf16, kind="ExternalInput")
    z_out = nc.dram_tensor("z_out", [MT, N_MTILES], f32, kind="ExternalOutput")
    z2_out = nc.dram_tensor("z2_out", [MT, NP_LOC], bf16, kind="ExternalOutput")

    MIN = mybir.AluOpType.min
    AX = mybir.AxisListType.X

    with tile.TileContext(nc) as tc:
        with (
            tc.tile_pool(name="stat", bufs=1) as stat_pool,
            tc.tile_pool(name="psum", bufs=2, space="PSUM") as psum_pool,
            tc.tile_pool(name="cp", bufs=3) as cp_pool,
            tc.tile_pool(name="tr", bufs=3) as tr_pool,
        ):
            stat_sb = stat_pool.tile([K, NG + NP_LOC], bf16)
            nc.sync.dma_start(out=stat_sb, in_=stat_in[:, :])
            gt_sb = stat_sb[:, 0:NG]
            pr_sb = stat_sb[:, NG : NG + NP_LOC]

            z2_acc = stat_pool.tile([MT, NP_LOC], bf16)
            nc.vector.memset(z2_acc, BIG)
            z_mins = stat_pool.tile([MT, N_MTILES], f32)

            for i in range(N_MTILES):
                cp = cp_pool.tile([MT, NP_LOC], bf16)
                for h in range(2):
                    bigps = psum_pool.tile([MT, HALF, NB], f32, tag="big")
                    for q in range(HALF):
                        nc.tensor.matmul(
                            bigps[:, q, :],
                            gt_sb[:, ts(i, MT)],
                            pr_sb[:, ts(4 * h + q, NB)],
                            start=True, stop=True,
                        )
                    nc.scalar.copy(
                        cp[:, 2048 * h : 2048 * (h + 1)],
                        bigps.rearrange("p a b -> p (a b)"),
                    )

                nc.vector.tensor_tensor(z2_acc, cp, z2_acc, op=MIN)

                t1 = tr_pool.tile([MT, 2048], bf16, tag="t1")
                nc.vector.tensor_tensor(t1, cp[:, 0:2048], cp[:, 2048:4096], op=MIN)
                t2 = tr_pool.tile([MT, 1024], bf16, tag="t2")
                nc.vector.tensor_tensor(t2, t1[:, 0:1024], t1[:, 1024:2048], op=MIN)
                t3 = tr_pool.tile([MT, 512], bf16, tag="t3")
                nc.vector.tensor_tensor(t3, t2[:, 0:512], t2[:, 512:1024], op=MIN)
                g, r = divmod(i, RGRP)
                if r == 0:
                    t4buf = tr_pool.tile([MT, RGRP, 256], bf16, tag="t4buf")
                nc.vector.tensor_tensor(
                    t4buf[:, r, :], t3[:, 0:256], t3[:, 256:512], op=MIN
                )
                if r == RGRP - 1:
                    nc.vector.tensor_reduce(
                        z_mins[:, g * RGRP : (g + 1) * RGRP],
                        t4buf,
                        axis=AX,
                        op=MIN,
                    )

            nc.sync.dma_start(out=z_out[:, :], in_=z_mins)
            nc.sync.dma_start(out=z2_out[:, :], in_=z2_acc)

    nc.compile()
    _CACHE["nc"] = nc
    return nc


def _split3(x):
    import ml_dtypes

    x1 = x.astype(ml_dtypes.bfloat16).astype(np.float32)
    r = x - x1
    x2 = r.astype(ml_dtypes.bfloat16).astype(np.float32)
    x3 = (r - x2).astype(ml_dtypes.bfloat16).astype(np.float32)
    return x1, x2, x3


def _prep_core_inputs(predict_pc, gt_pc, c):
    import ml_dtypes

    b, h = divmod(c, 2)
    P = predict_pc[b][:, h * NP_LOC : (h + 1) * NP_LOC].astype(np.float32)
    G = gt_pc[b].astype(np.float32)
    g2 = (G * G).sum(axis=0)
    p2 = (P * P).sum(axis=0)
    G1, G2s, G3 = _split3(G)
    P1, P2s, P3 = _split3(-2.0 * P)
    g21, g22, g23 = _split3(g2)
    p21, p22, p23 = _split3(p2)
    ones_g = np.ones((1, NG), np.float32)
    ones_p = np.ones((1, NP_LOC), np.float32)
    # pairs (i,j) of splits kept: (1,1),(1,2),(2,1),(1,3),(3,1),(2,2)
    gt_rows = [G1, G1, G2s, G1, G3, G2s,
               ones_g, ones_g, ones_g, g21[None], g22[None], g23[None]]
    pr_rows = [P1, P2s, P1, P3, P1, P2s,
               p21[None], p22[None], p23[None], ones_p, ones_p, ones_p]
    gt_stat = np.concatenate(gt_rows, axis=0)   # [6*3 + 6, NG] = [24, NG]
    pr_mov = np.concatenate(pr_rows, axis=0)
    stat = np.concatenate([gt_stat, pr_mov], axis=1)
    assert stat.shape == (K, NG + NP_LOC)
    return {"stat_in": np.ascontiguousarray(stat.astype(ml_dtypes.bfloat16))}


def kernel(predict_pc, gt_pc):
    global LAST_EXEC_NS
    predict_pc = np.asarray(predict_pc, dtype=np.float32)
    gt_pc = np.asarray(gt_pc, dtype=np.float32)

    nc = _build()
    in_maps = [_prep_core_inputs(predict_pc, gt_pc, c) for c in range(N_CORES)]
    trace = os.environ.get("CHAMFER_TRACE", "0") == "1"
    res = run_bass_kernel_spmd(
        nc, in_maps, core_ids=list(range(N_CORES)), trace=trace
    )
    LAST_EXEC_NS = res.exec_time_ns

    denom = B * (NG + NP_FULL)
    z_sum = 0.0
    z2_sum = 0.0
    for b in range(B):
        r0 = res.results[2 * b]
        r1 = res.results[2 * b + 1]
        zmin = np.minimum(r0["z_out"], r1["z_out"])
        z_sum += np.sqrt(np.maximum(zmin, EPS)).sum(dtype=np.float64)
        for r in (r0, r1):
            z2 = r["z2_out"].astype(np.float32).min(axis=0)
            z2_sum += np.sqrt(np.maximum(z2.astype(np.float64), EPS)).sum()
    loss = (z_sum + z2_sum) / denom
    return np.float32(loss)
